# revision 1
# baseline (speedup 1.0000x reference)
"""AdjSAGE (3-layer GraphSAGE, mean aggregation) on 8 Trainium2 NeuronCores.

Strategy (graph/data parallel, per the dst-partition sharding):
  - Nodes are partitioned by destination across the 8 cores (12500 dst/core).
  - Per layer L we gather y_L = h_{L-1} @ Wl_L.T rows by edge src (indirect
    DMA, 512B rows), then segment-sum into dst rows on the PE array using
    one-hot selection matmuls (Sel.T @ G), scale by 1/deg, add the root term
    h_{L-1} @ Wr_L.T (dense matmul against the SBUF-resident transposed own
    shard), bias, ReLU.  y_{L+1} shards are AllGathered into a replicated
    HBM copy that serves as the next layer's gather source.
  - Edge index streams / selection metadata are precomputed host-side from
    edge_src/edge_dst (graph structure only) and fed as per-core inputs.
    The SPMD program is shared by all 8 cores, so per-group slot quotas are
    maxed across cores and padded (pad slots gather row 0 and carry a -1
    dst tag so they contribute nothing).
"""

import os
import sys

for _p in ("/opt/trn_rl_repo", "/root/.axon_site/_ro/trn_rl_repo"):
    if os.path.isdir(_p) and _p not in sys.path:
        sys.path.insert(0, _p)

import numpy as np

import concourse.bacc as bacc
import concourse.tile as tile
from concourse import mybir
from concourse.bass_utils import run_bass_kernel_spmd

# Problem shape (nn_AdjSAGE_23596550324897)
N = 100000
E = 1600000
D = 128
DOUT = 40
DOUTP = 64          # padded output feature width (256B gather rows)
NCORES = 8
NPC = N // NCORES   # 12500 dst nodes per core
TILE = 128
NT = (NPC + TILE - 1) // TILE   # 98 tiles (last has 84 rows)
SG = 4                           # tiles per supergroup (one PSUM bank)
NG = (NT + SG - 1) // SG         # 25 supergroups
CH = 4                           # gather-source row chunks (int16 idx limit)
CKS = NPC // CH                  # 3125 rows per core per chunk
CHROWS = NCORES * CKS            # 25000 rows per chunk tensor
F32 = mybir.dt.float32
I16 = mybir.dt.int16
GATHER_MAX = 1024   # >1024-idx dma_gather calls fail on HW (SWDGE ring limit)

_CACHE = {}


def _tiles_in(g):
    return min(SG, NT - g * SG)


def _structure(edge_src, edge_dst):
    """Host-side preprocessing: per-core slot streams + shared quotas."""
    edge_src = np.asarray(edge_src).astype(np.int64)
    edge_dst = np.asarray(edge_dst).astype(np.int64)
    deg = np.bincount(edge_dst, minlength=N)
    invdeg = (1.0 / np.maximum(deg, 1)).astype(np.float32)

    # group = (supergroup g, src chunk k, tile-in-supergroup tl); calls are
    # padded at (g, k) granularity only — a 128-slot scatter chunk may then
    # straddle tile boundaries, handled by per-(chunk, tile) segments whose
    # set is the union over cores (shared SPMD program).
    GI = NG * CH * SG
    NCALL = NG * CH
    counts = np.zeros((NCORES, GI), np.int64)
    percore = []
    for c in range(NCORES):
        m = (edge_dst >= c * NPC) & (edge_dst < (c + 1) * NPC)
        src = edge_src[m]
        dl = edge_dst[m] - c * NPC
        t = dl >> 7
        # chunk k of a source node: which quarter of its owner's shard it
        # falls in; chunk tensor row = owner*CKS + (local % CKS)
        k = (src % NPC) // CKS
        gi = ((t // SG) * CH + k) * SG + (t % SG)
        # secondary sort by dst: narrow per-chunk dst windows (32-wide
        # Sel segments) matter more than gather address locality
        order = np.lexsort((dl, gi))
        counts[c] = np.bincount(gi, minlength=GI)
        percore.append((gi[order], src[order], dl[order]))

    callcnt = counts.reshape(NCORES, NCALL, SG).sum(2)
    QC = ((callcnt.max(0) + 127) // 128) * 128          # per-call slot quota
    QCstart = np.concatenate(([0], np.cumsum(QC)))
    S = int(QC.sum())
    nch_call = QC // 128

    # segment sets: union over cores of occupied (chunk j, tl, 32-block b)
    MAXJ = 64
    NB = TILE // 32
    touch = np.zeros((NCALL, MAXJ, SG, NB), bool)
    pcdata = []
    for c in range(NCORES):
        gi_s, src_s, dl_s = percore[c]
        ci_s = gi_s // SG
        starts = np.concatenate(
            ([0], np.cumsum(np.bincount(ci_s, minlength=NCALL))))[:-1]
        pos = np.arange(gi_s.size) - starts[ci_s]
        j_s = pos // 128
        b_s = (dl_s & 127) // 32
        touch[ci_s, j_s, gi_s % SG, b_s] = True
        pcdata.append((ci_s, pos, j_s, b_s))
    segs = []               # per call: list of (j, tl, b)
    segcol = np.full(NCALL * MAXJ * SG * NB, -1, np.int64)
    nsegtot = 0
    for ci in range(NCALL):
        lst = [(j, tl, b) for j in range(int(nch_call[ci]))
               for tl in range(SG) for b in range(NB)
               if touch[ci, j, tl, b]]
        segs.append(lst)
        for (j, tl, b) in lst:
            segcol[((ci * MAXJ + j) * SG + tl) * NB + b] = nsegtot
            nsegtot += 1
    NSEG = (nsegtot + 127) // 128 * 128                 # pad for tidy DMA

    idx16s, dstlocs, invdegs = [], [], []
    for c in range(NCORES):
        gi_s, src_s, dl_s = percore[c]
        ci_s, pos, j_s, b_s = pcdata[c]
        slot = QCstart[ci_s] + pos
        idxval = ((src_s // NPC) * CKS + (src_s % NPC) % CKS).astype(np.int16)
        assert idxval.min() >= 0 and int(idxval.max()) < CHROWS

        idx_flat = np.zeros(S, np.int16)
        idx_flat[slot] = idxval

        col = segcol[((ci_s * MAXJ + j_s) * SG + (gi_s % SG)) * NB + b_s]
        assert col.min() >= 0
        dl128 = np.full((128, NSEG), -1.0, np.float32)
        dl128[pos % 128, col] = ((dl_s & 127) - b_s * 32).astype(np.float32)

        idx16 = np.tile(idx_flat.reshape(S // 16, 16).T, (8, 1))

        iv = np.ones(NT * TILE, np.float32)
        iv[:NPC] = invdeg[c * NPC:(c + 1) * NPC]
        idx16s.append(np.ascontiguousarray(idx16))
        dstlocs.append(dl128)
        invdegs.append(np.ascontiguousarray(iv.reshape(NT, TILE).T))

    return {
        "QC": QC, "S": S, "NSEG": NSEG, "segs": segs,
        "idx16": idx16s, "dstloc": dstlocs, "invdeg": invdegs,
        "deg": deg,
    }


def _build(QC, S, NSEG, segs):
    """Emit the shared SPMD Bass program (structure shared by all cores)."""
    nswq = int(os.environ.get("K_NSWQ", "1"))
    nc = bacc.Bacc("TRN2", target_bir_lowering=False, debug=False,
                   num_devices=NCORES, num_swdge_queues=nswq)

    xsh = nc.dram_tensor("xsh", [NPC, D], F32, kind="ExternalInput")
    idx_in = nc.dram_tensor("idx16", [128, S // 16], I16, kind="ExternalInput")
    dl_in = nc.dram_tensor("dstloc", [128, NSEG], F32, kind="ExternalInput")
    iv_in = nc.dram_tensor("invdeg", [128, NT], F32, kind="ExternalInput")
    w_in = {}
    for nm, cols in (("WlT0", D), ("WrT0", D), ("WlT1", D), ("WrT1", D),
                     ("WlT2", DOUTP), ("WrT2", DOUTP),
                     ("b0r", D), ("b1r", D), ("b2r", DOUTP)):
        w_in[nm] = nc.dram_tensor(nm, [128, cols], F32, kind="ExternalInput")
    id_in = nc.dram_tensor("ident", [128, 128], F32, kind="ExternalInput")
    out_ext = nc.dram_tensor("out", [NPC, DOUT], F32, kind="ExternalOutput")

    EL0 = [D, D, DOUTP]
    yfull = [
        [nc.dram_tensor(f"y{L}f{k}", [CHROWS, EL0[L]], F32,
                        addr_space="Shared") for k in range(CH)]
        for L in range(3)
    ]
    # last tile covering each source chunk (collective fires after it)
    AG_TILE = [((k + 1) * CKS + TILE - 1) // TILE - 1 for k in range(CH)]

    _build._gq = 0
    nchmax = max(1, int(QC.max()) // 128)
    nsegmax = max(len(l) for l in segs)

    EL = [D, D, DOUTP]  # gather row width per layer

    with tile.TileContext(nc) as tc:
        with (
            tc.tile_pool(name="const", bufs=1) as const,
            tc.tile_pool(name="xrow", bufs=3) as xpool,
            tc.tile_pool(name="gbuf", bufs=3) as gpool,
            tc.tile_pool(name="selbuf", bufs=2) as selpool,
            tc.tile_pool(name="ybuf", bufs=3) as ypool,
            tc.tile_pool(name="small", bufs=4) as small,
            tc.tile_pool(name="stat", bufs=4) as stat,
            tc.tile_pool(name="psg", bufs=2, space="PSUM") as sgp,
            tc.tile_pool(name="ptp", bufs=2, space="PSUM") as tpp,
            tc.tile_pool(name="prr", bufs=2, space="PSUM") as rrp,
            tc.tile_pool(name="pyy", bufs=2, space="PSUM") as yyp,
            tc.tile_pool(name="dram", bufs=1, space="DRAM") as dram,
        ):
            # ---- resident constants ----
            idx_sb = const.tile([128, S // 16], I16)
            nc.sync.dma_start(idx_sb[:], idx_in[:])
            dl_sb = const.tile([128, NSEG], F32)
            nc.sync.dma_start(dl_sb[:], dl_in[:])
            iv_sb = const.tile([128, NT], F32)
            nc.sync.dma_start(iv_sb[:], iv_in[:])
            w_sb = {}
            for nm, t_in in w_in.items():
                w_sb[nm] = const.tile(list(t_in.shape), F32, name=f"w_{nm}")
                nc.sync.dma_start(w_sb[nm][:], t_in[:])
            id_sb = const.tile([128, 128], F32)
            nc.sync.dma_start(id_sb[:], id_in[:])
            iota = const.tile([128, nsegmax * 32], F32)
            nc.gpsimd.iota(
                iota[:].rearrange("p (c w) -> p c w", w=32),
                [[0, nsegmax], [1, 32]], channel_multiplier=0,
                allow_small_or_imprecise_dtypes=True,
            )
            hT = const.tile([128, NT * 128], F32)   # transposed own-shard acts

            stg = [
                [dram.tile([CKS, EL0[L]], F32, name=f"st{L}_{k}")
                 for k in range(CH)]
                for L in range(3)
            ]

            def y_write(L, t, rows, ysb):
                i0 = t * TILE
                for k in range(i0 // CKS, (i0 + rows - 1) // CKS + 1):
                    lo = max(i0, k * CKS)
                    hi = min(i0 + rows, (k + 1) * CKS)
                    nc.sync.dma_start(
                        stg[L][k][lo - k * CKS:hi - k * CKS, :],
                        ysb[lo - i0:hi - i0, :])

            def emit_ag(L, k):
                nc.gpsimd.collective_compute(
                    "AllGather", mybir.AluOpType.bypass,
                    replica_groups=[list(range(NCORES))],
                    ins=[stg[L][k][:]], outs=[yfull[L][k][:]],
                )

            # ---- prologue: hT = x.T tiles; y0 = x @ Wl0.T; AllGather ----
            for t in range(NT):
                rows = min(TILE, NPC - t * TILE)
                xr = xpool.tile([128, D], F32)
                nc.sync.dma_start(xr[:rows, :], xsh[t * TILE:t * TILE + rows, :])
                ptp = tpp.tile([128, 128], F32)
                nc.tensor.transpose(ptp[:], xr[:], id_sb[:])
                nc.scalar.activation(hT[:, t * 128:(t + 1) * 128], ptp[:],
                                     mybir.ActivationFunctionType.Copy)
                py = yyp.tile([128, D], F32)
                nc.tensor.matmul(py[:], hT[:, t * 128:(t + 1) * 128],
                                 w_sb["WlT0"][:], start=True, stop=True)
                ysb = ypool.tile([128, D], F32)
                nc.scalar.activation(ysb[:], py[:],
                                     mybir.ActivationFunctionType.Copy)
                y_write(0, t, rows, ysb)
                for k in range(CH):
                    if AG_TILE[k] == t:
                        emit_ag(0, k)

            # ---- layers ----
            n_layers = int(os.environ.get("K_NL", "3"))
            if n_layers == 0:
                # debug: dump y0 head to out
                dbg = ypool.tile([128, DOUT], F32, tag="dbg")
                for t in range(NT):
                    rows = min(TILE, NPC - t * TILE)
                    nc.sync.dma_start(dbg[:rows, :],
                                      yfull[0][0][t * TILE:t * TILE + rows, :DOUT])
                    nc.sync.dma_start(out_ext[t * TILE:t * TILE + rows, :],
                                      dbg[:rows, :])
            parts = int(os.environ.get("K_PARTS", "15"))
            ng_lim = int(os.environ.get("K_NG", str(NG)))
            for L in range(n_layers):
                el = EL[L]
                wl_next = ("WlT1", "WlT2", None)[L]
                wr = w_sb[("WrT0", "WrT1", "WrT2")[L]]
                br = w_sb[("b0r", "b1r", "b2r")[L]]
                cs = 0   # slot offset
                dc = 0   # dstloc/segment column offset
                for g in range(NG):
                    if g >= ng_lim:
                        break
                    ntl = _tiles_in(g)
                    psg = sgp.tile([128, SG * el], F32)
                    mms = []  # (tl, sel, gt, j, segcol)
                    for k in range(CH):
                        ci = g * CH + k
                        sz = int(QC[ci])
                        lst = segs[ci]
                        if sz == 0:
                            continue
                        nch = sz // 128
                        gt = gpool.tile([128, nchmax * el], F32, tag="G")
                        if parts & 1:
                            gv = gt[:, :nch * el].rearrange(
                                "p (c e) -> p c e", e=el)
                            for s0 in range(0, sz, GATHER_MAX):
                                ssz = min(GATHER_MAX, sz - s0)
                                nc.gpsimd.dma_gather(
                                    gv[:, s0 // 128:(s0 + ssz) // 128, :],
                                    yfull[L][k][:, :],
                                    idx_sb[:, (cs + s0) // 16:
                                           (cs + s0 + ssz) // 16],
                                    ssz, ssz, el,
                                    queue_num=_build._gq % nswq,
                                )
                                _build._gq += 1
                        nseg = len(lst)
                        sel = selpool.tile([128, nsegmax * 32], F32, tag="S")
                        if parts & 2:
                            nc.vector.tensor_tensor(
                                sel[:, :nseg * 32].rearrange(
                                    "p (c w) -> p c w", w=32),
                                iota[:, :nseg * 32].rearrange(
                                    "p (c w) -> p c w", w=32),
                                dl_sb[:, dc: dc + nseg]
                                    .unsqueeze(2).broadcast_to([128, nseg, 32]),
                                mybir.AluOpType.is_equal,
                            )
                        for si, (j, tl, b) in enumerate(lst):
                            mms.append((tl, b, sel, gt, j, si))
                        cs += sz
                        dc += nseg
                    if parts & 4:
                        first_b = {}
                        last_b = {}
                        for i, (tl, b, sel, gt, j, si) in enumerate(mms):
                            first_b.setdefault(b, i)
                            last_b[b] = i
                        for i, (tl, b, sel, gt, j, si) in enumerate(mms):
                            nc.tensor.matmul(
                                psg[b * 32:(b + 1) * 32,
                                    tl * el:(tl + 1) * el],
                                sel[:, si * 32:(si + 1) * 32],
                                gt[:, j * el:(j + 1) * el],
                                start=(first_b[b] == i),
                                stop=(last_b[b] == i),
                                tile_position=(0, b * 32),
                            )
                    if not (parts & 8):
                        continue
                    # per-tile epilogue
                    for tl in range(ntl):
                        t = g * SG + tl
                        rows = min(TILE, NPC - t * TILE)
                        agg = small.tile([128, el], F32, tag="agg")
                        nc.vector.tensor_scalar(
                            agg[:], psg[:, tl * el:(tl + 1) * el],
                            iv_sb[:, t:t + 1], None, mybir.AluOpType.mult)
                        pr = rrp.tile([128, el], F32)
                        nc.tensor.matmul(pr[:], hT[:, t * 128:(t + 1) * 128],
                                         wr[:], start=True, stop=True)
                        t2 = small.tile([128, el], F32, tag="t2")
                        nc.vector.tensor_tensor(t2[:], agg[:], pr[:],
                                                mybir.AluOpType.add)
                        t3 = small.tile([128, el], F32, tag="t3")
                        nc.vector.tensor_tensor(t3[:], t2[:], br[:],
                                                mybir.AluOpType.add)
                        if L < 2:
                            hs = small.tile([128, el], F32, tag="hs")
                            nc.scalar.activation(hs[:], t3[:],
                                                 mybir.ActivationFunctionType.Relu)
                            ptp = tpp.tile([128, 128], F32)
                            nc.tensor.transpose(ptp[:], hs[:], id_sb[:])
                            nc.scalar.activation(hT[:, t * 128:(t + 1) * 128],
                                                 ptp[:],
                                                 mybir.ActivationFunctionType.Copy)
                            eln = EL[L + 1]
                            py = yyp.tile([128, eln], F32, tag="py")
                            nc.tensor.matmul(py[:], hT[:, t * 128:(t + 1) * 128],
                                             w_sb[wl_next][:], start=True,
                                             stop=True)
                            ysb = ypool.tile([128, eln], F32, tag="ys")
                            nc.scalar.activation(ysb[:], py[:],
                                                 mybir.ActivationFunctionType.Copy)
                            y_write(L + 1, t, rows, ysb)
                            for kk in range(CH):
                                if AG_TILE[kk] == t:
                                    emit_ag(L + 1, kk)
                        else:
                            mx = stat.tile([128, 1], F32, tag="mx")
                            nc.vector.tensor_reduce(
                                mx[:], t3[:, :DOUT], mybir.AxisListType.X,
                                mybir.AluOpType.max, negate=True)
                            ex = small.tile([128, DOUT], F32, tag="ex")
                            ssum = stat.tile([128, 1], F32, tag="ss")
                            nc.scalar.activation(
                                ex[:], t3[:, :DOUT],
                                mybir.ActivationFunctionType.Exp,
                                bias=mx[:], accum_out=ssum[:])
                            ls = stat.tile([128, 1], F32, tag="ls")
                            nc.scalar.activation(
                                ls[:], ssum[:], mybir.ActivationFunctionType.Ln)
                            tot = stat.tile([128, 1], F32, tag="tot")
                            nc.vector.tensor_tensor(
                                tot[:], mx[:], ls[:], mybir.AluOpType.subtract)
                            osb = small.tile([128, DOUT], F32, tag="os")
                            nc.vector.tensor_scalar(
                                osb[:], t3[:, :DOUT], tot[:], None,
                                mybir.AluOpType.add)
                            nc.sync.dma_start(
                                out_ext[t * TILE:t * TILE + rows, :],
                                osb[:rows, :])
            if 0 < n_layers < 3:
                dbg2 = ypool.tile([128, DOUT], F32, tag="dbg")
                for t in range(NT):
                    rows = min(TILE, NPC - t * TILE)
                    nc.sync.dma_start(
                        dbg2[:rows, :],
                        yfull[n_layers][0][t * TILE:t * TILE + rows, :DOUT])
                    nc.sync.dma_start(out_ext[t * TILE:t * TILE + rows, :],
                                      dbg2[:rows, :])
    nc.compile()
    return nc


def _prepare(inputs):
    edge_src = np.asarray(inputs["edge_src"])
    edge_dst = np.asarray(inputs["edge_dst"])
    key = (hash(edge_src.tobytes()) ^ hash(edge_dst.tobytes()))
    if key in _CACHE:
        return _CACHE[key]
    st = _structure(edge_src, edge_dst)
    nc = _build(st["QC"], st["S"], st["NSEG"], st["segs"])
    _CACHE[key] = (st, nc)
    return st, nc


def _make_in_maps(st, inputs):
    x = np.asarray(inputs["x"], np.float32)

    def wt(a):
        return np.ascontiguousarray(np.asarray(a, np.float32).T)

    WlT2 = np.zeros((D, DOUTP), np.float32)
    WlT2[:, :DOUT] = wt(inputs["Wl2"])
    WrT2 = np.zeros((D, DOUTP), np.float32)
    WrT2[:, :DOUT] = wt(inputs["Wr2"])
    b2r = np.zeros((128, DOUTP), np.float32)
    b2r[:, :DOUT] = np.tile(np.asarray(inputs["b2"], np.float32), (128, 1))

    shared = {
        "WlT0": wt(inputs["Wl0"]), "WrT0": wt(inputs["Wr0"]),
        "WlT1": wt(inputs["Wl1"]), "WrT1": wt(inputs["Wr1"]),
        "WlT2": WlT2, "WrT2": WrT2,
        "b0r": np.tile(np.asarray(inputs["b0"], np.float32), (128, 1)),
        "b1r": np.tile(np.asarray(inputs["b1"], np.float32), (128, 1)),
        "b2r": b2r,
        "ident": np.eye(128, dtype=np.float32),
    }
    in_maps = []
    for c in range(NCORES):
        m = dict(shared)
        m["xsh"] = np.ascontiguousarray(x[c * NPC:(c + 1) * NPC])
        m["idx16"] = st["idx16"][c]
        m["dstloc"] = st["dstloc"][c]
        m["invdeg"] = st["invdeg"][c]
        in_maps.append(m)
    return in_maps


def kernel(**inputs):
    st, nc = _prepare(inputs)
    res = run_bass_kernel_spmd(nc, _make_in_maps(st, inputs),
                               list(range(NCORES)))
    out = np.concatenate([res.results[c]["out"] for c in range(NCORES)], axis=0)
    return out.astype(np.float32)


if __name__ == "__main__":
    sys.path.insert(0, os.path.dirname(os.path.abspath(__file__)))
    import reference
    inputs = {k: np.asarray(v) for k, v in reference.setup_inputs().items()}
    got = kernel(**inputs)
    want = np.asarray(reference.reference(**reference.setup_inputs()))
    err = np.abs(got - want).max() / np.abs(want).max()
    print("Relative error:", err)

